# revision 13
# baseline (speedup 1.0000x reference)
"""Trainium2 Bass kernel for nn_Attn_14078902796904.

Computes attn = softmax(encoder_outputs @ hidden) for
encoder_outputs [65536, 1024] f32, hidden [1024] f32 -> [1, 1, 65536] f32.

Strategy (sequence-parallel across 8 NeuronCores):
  - Core c gets rows [c*8192, (c+1)*8192) of encoder_outputs; hidden is
    replicated (host pre-broadcasts it to [128, 1024]).
  - On-core: stream the 32 MB shard through SBUF in [128, nb*1024]
    chunks with a CONTIGUOUS per-partition layout (partition p of a
    chunk holds nb consecutive rows -> 16 KB contiguous HBM reads per
    partition, 16 KB DMA descriptors).
  - SDMA engine 15 (serving SBUF partitions 92-95/124-127) intermittently
    runs ~20% slower than the other 15 engines and then paces the whole
    stream (observed on most runs).  Mitigation: partition skew -- those
    8 partitions receive only 49 rows while the 120 fast partitions get
    65, via 12 all-partition chunks + 5 fast-partition-only chunks + a
    final all-partition 1-block chunk.  Engine 15 carries 23% fewer
    bytes, so the stream stays ~engine-balanced whether or not it is
    degraded.
  - Compute: the DVE multiplies each chunk by hid IN PLACE (f32
    tensor_tensor is 1 elem/lane/cycle); per-block row-sums are split
    between the Scalar engine (activation Identity + accum_out) and the
    DVE (reduce_sum) -- scalar takes most, DVE takes a few mid-stream
    ones plus the final block so neither engine holds a backlog when
    the stream ends.
  - Device returns raw energies; softmax runs on the host in float64
    over all 65536 gathered energies.
  (tensor_tensor_reduce would fuse mul+reduce in one DVE op but crashes
  the execution unit on this runtime path -- probed 2026-08-08.)
"""

import os
import sys
import time

for _p in ("/opt/trn_rl_repo", "/root/.axon_site/_ro/trn_rl_repo"):
    if os.path.isdir(_p) and _p not in sys.path:
        sys.path.append(_p)

import numpy as np

import concourse.tile as tile
from concourse import bacc, mybir
from concourse.bass_utils import run_bass_kernel_spmd

S = 65536
H = 1024
N_CORES = 8
SC = S // N_CORES          # 8192 rows per core
P = 128                    # partitions
GMAX = 4                   # max blocks per DMA chunk (2 MB; 8-block
                           # chunks hit a DVE efficiency cliff)

# chunk plan: ('a', nb) = all 128 partitions get nb rows;
#             ('f', nb) = only the 120 fast partitions get nb rows.
# all-partition rows: 12*4 + 1 = 49; fast-only adds 16 -> 65 rows.
# 128*49 + ... total = 120*65 + 8*49 = 8192 rows.
CHUNKS = [('a', 4)] * 12 + [('f', 4)] * 3 + [('f', 2)] * 2 + [('a', 1)]
NF = 120                   # fast partitions (all but 92-95, 124-127)
assert 128 * sum(nb for k, nb in CHUNKS if k == 'a') + \
    NF * sum(nb for k, nb in CHUNKS if k == 'f') == SC

# energies column index ranges per chunk (one column per block)
_COLS = []
_c = 0
for _k, _nb in CHUNKS:
    _COLS.append(_c)
    _c += _nb
NCOLS = _c                 # 65

INP_BUFS = 6

_DT = mybir.dt.float32


def _vector_blocks(g):
    """Block offsets (within chunk g) whose row-sum runs on the DVE;
    the rest run on the Scalar engine.  The DVE carries all the
    multiplies, so it only takes a few mid-stream reduces plus the
    final block (it is the engine that finishes the last mul)."""
    k, nb = CHUNKS[g]
    if g == len(CHUNKS) - 1:
        return [0]
    if nb == 4 and g in (1, 3, 5, 7, 9):
        return [3]
    return []


def _fast_rows(p):
    """Tile partition p -> fast-partition index (None for slow)."""
    if p < 92:
        return p
    if 96 <= p < 124:
        return p - 4
    return None


def _build_nc():
    nc = bacc.Bacc("TRN2", target_bir_lowering=False, debug=False,
                   enable_asserts=False, num_devices=N_CORES)
    enc = nc.dram_tensor("enc", [SC, H], _DT, kind="ExternalInput")
    hid = nc.dram_tensor("hid", [P, H], _DT, kind="ExternalInput")
    n_v = sum(len(_vector_blocks(g)) for g in range(len(CHUNKS)))
    out_s = nc.dram_tensor("out_s", [P, NCOLS], _DT, kind="ExternalOutput")
    out_v = nc.dram_tensor("out_v", [P, n_v], _DT, kind="ExternalOutput")

    with tile.TileContext(nc) as tc:
        with (
            tc.tile_pool(name="inp", bufs=INP_BUFS) as inp_pool,
            tc.tile_pool(name="small", bufs=1) as small,
        ):
            hidrep = small.tile([P, H], _DT)

            # separate tiles per engine so the Tile tracker never makes
            # one engine's energy writes wait on the other's
            energies_s = small.tile([P, NCOLS], _DT)
            energies_v = small.tile([P, max(n_v, 1)], _DT)
            # DVE-reduced columns of energies_s are host-overwritten;
            # zero once so the out_s DMA reads initialized memory
            nc.gpsimd.memset(energies_s[:], 0.0)

            row = 0            # next unassigned HBM row of this shard
            vcol = 0
            for g, (kind, nb) in enumerate(CHUNKS):
                t_in = inp_pool.tile([P, GMAX * H], _DT, tag="t_in")
                if kind == 'a':
                    # partition p <- rows [row + p*nb, row + (p+1)*nb)
                    nc.sync.dma_start(
                        t_in[:, :nb * H].rearrange("p (b h) -> p b h", h=H),
                        enc.ap()[row:row + nb * P, :].rearrange(
                            "(p b) h -> p b h", p=P),
                    )
                    row += nb * P
                    pn = P
                else:
                    # fast partitions only: tile partitions [0:92] and
                    # [96:124]; engine 15's partitions get nothing.
                    # Zero [64:96] first (aligned superset of the 92-95
                    # hole) so the full-width ops below read initialized
                    # memory; the DMA overwrites [64:92] right after.
                    nc.gpsimd.memset(t_in[64:96, :nb * H], 0.0)
                    nc.sync.dma_start(
                        t_in[0:92, :nb * H].rearrange(
                            "p (b h) -> p b h", h=H),
                        enc.ap()[row:row + nb * 92, :].rearrange(
                            "(p b) h -> p b h", p=92),
                    )
                    nc.sync.dma_start(
                        t_in[96:124, :nb * H].rearrange(
                            "p (b h) -> p b h", h=H),
                        enc.ap()[row + nb * 92:row + nb * 120, :].rearrange(
                            "(p b) h -> p b h", p=28),
                    )
                    # partitions 92-95 keep the previous chunk's stale
                    # (finite) data -- the full-width ops below compute
                    # garbage there, which the host ignores.  The first
                    # INP_BUFS chunks are all-partition, so the buffer
                    # is always fully initialized.
                    row += nb * NF
                    pn = 124
                if g == 0:
                    # issued after chunk 0 so the big stream's first
                    # byte isn't delayed behind the hid replica
                    nc.sync.dma_start(hidrep[:], hid.ap())

                hid_bc = hidrep[0:pn].rearrange(
                    "p (o h) -> p o h", o=1).broadcast_to((pn, nb, H))
                nc.vector.tensor_mul(
                    t_in[0:pn, :nb * H].rearrange("p (b h) -> p b h", h=H),
                    t_in[0:pn, :nb * H].rearrange("p (b h) -> p b h", h=H),
                    hid_bc,
                )
                vblocks = _vector_blocks(g)
                for j in range(nb):
                    seg = t_in[0:pn, j * H:(j + 1) * H]
                    if j in vblocks:
                        nc.vector.reduce_sum(
                            energies_v[0:pn, vcol:vcol + 1], seg,
                            axis=mybir.AxisListType.X,
                        )
                        vcol += 1
                    else:
                        col = _COLS[g] + j
                        nc.scalar.activation(
                            seg, seg,
                            mybir.ActivationFunctionType.Identity,
                            accum_out=energies_s[0:pn, col:col + 1],
                        )

            nc.sync.dma_start(out_s.ap(), energies_s[:])
            nc.sync.dma_start(out_v.ap(), energies_v[:, :n_v])
    nc.compile()
    return nc


_NC_CACHE = None


def _get_nc():
    global _NC_CACHE
    if _NC_CACHE is None:
        _NC_CACHE = _build_nc()
    return _NC_CACHE


def run_device(hidden, encoder_outputs, **spmd_kwargs):
    """Run the per-core kernels; returns (list of per-core result dicts,
    BassKernelResults)."""
    hidden = np.asarray(hidden, dtype=np.float32)
    encoder_outputs = np.asarray(encoder_outputs, dtype=np.float32)
    hidrep = np.ascontiguousarray(np.broadcast_to(hidden, (P, H)))
    in_maps = [
        {
            "enc": np.ascontiguousarray(encoder_outputs[c * SC:(c + 1) * SC]),
            "hid": hidrep,
        }
        for c in range(N_CORES)
    ]
    # The axon-proxied runtime occasionally reports the accelerator as
    # unrecoverable and then recovers on the next attempt; retry.
    last_err = None
    for attempt in range(3):
        try:
            res = run_bass_kernel_spmd(
                _get_nc(), in_maps, list(range(N_CORES)), **spmd_kwargs
            )
            return res.results, res
        except Exception as e:  # noqa: BLE001
            last_err = e
            time.sleep(2.0)
    raise last_err


def _maps():
    """(vcols, perm): vcols[i] = energies column of the i-th DVE reduce;
    perm[s_local] = flat index into the merged [P, NCOLS] energies."""
    vcols = []
    for g, (kind, nb) in enumerate(CHUNKS):
        for j in _vector_blocks(g):
            vcols.append(_COLS[g] + j)
    perm = np.empty(SC, dtype=np.int64)
    row = 0
    for g, (kind, nb) in enumerate(CHUNKS):
        c0 = _COLS[g]
        if kind == 'a':
            for p in range(P):
                for j in range(nb):
                    perm[row + p * nb + j] = p * NCOLS + (c0 + j)
            row += nb * P
        else:
            for p in range(P):
                f = _fast_rows(p)
                if f is None:
                    continue
                for j in range(nb):
                    perm[row + f * nb + j] = p * NCOLS + (c0 + j)
            row += nb * NF
    return np.array(vcols), perm


_VCOLS, _PERM = _maps()


def combine(results):
    """Host-side softmax over the gathered energies -> [1, 1, S] f32."""
    es = []
    for r in results:
        E = r["out_s"].copy()
        E[:, _VCOLS] = r["out_v"]
        es.append(E.reshape(P * NCOLS)[_PERM])
    e = np.concatenate(es).astype(np.float64)
    e -= e.max()
    x = np.exp(e)
    attn = x / x.sum()
    return attn.astype(np.float32)[None, None, :]


def kernel(hidden, encoder_outputs):
    results, _ = run_device(hidden, encoder_outputs)
    return combine(results)


# revision 14
# speedup vs baseline: 1.1202x; 1.1202x over previous
"""Trainium2 Bass kernel for nn_Attn_14078902796904.

Computes attn = softmax(encoder_outputs @ hidden) for
encoder_outputs [65536, 1024] f32, hidden [1024] f32 -> [1, 1, 65536] f32.

Strategy (sequence-parallel across 8 NeuronCores):
  - Core c gets rows [c*8192, (c+1)*8192) of encoder_outputs; hidden is
    replicated (host pre-broadcasts it to [128, 1024]).
  - On-core: stream the 32 MB shard through SBUF in [128, nb*1024]
    chunks with a CONTIGUOUS per-partition layout (partition p of a
    chunk holds nb consecutive rows -> 16 KB contiguous HBM reads per
    partition, 16 KB DMA descriptors, ~405 GB/s observed).  Chunk DMAs
    alternate between the two HWDGE rings (sync / scalar queues).
    All chunks keep the full 128-partition shape: partition-sliced DMAs
    collapse onto 4 of the 16 SDMA rings (~1/4 bandwidth -- measured).
  - Compute: the DVE multiplies each chunk by hid IN PLACE (f32
    tensor_tensor is 1 elem/lane/cycle -> 4.42 us per 4-block chunk vs
    the 5.2-5.9 us DMA pace).  Per-block row-sums are split between the
    Scalar engine (activation Identity + accum_out, 1.22 us/block
    sustained) and the DVE (reduce_sum, 1.22 us/block): scalar takes
    most blocks, the DVE takes a few mid-stream ones plus the tapered
    tail blocks interleaved with its final muls, so neither engine
    holds a backlog when the stream ends.
  - Device returns raw energies; softmax runs on the host in float64
    over all 65536 gathered energies.  No on-device softmax -> no
    ACT_TABLE_LOAD, minimal post-stream tail.
  (tensor_tensor_reduce would fuse mul+reduce in one DVE op but crashes
  the execution unit on this runtime path -- probed 2026-08-08.)
"""

import os
import sys
import time

for _p in ("/opt/trn_rl_repo", "/root/.axon_site/_ro/trn_rl_repo"):
    if os.path.isdir(_p) and _p not in sys.path:
        sys.path.append(_p)

import numpy as np

import concourse.tile as tile
from concourse import bacc, mybir
from concourse.bass_utils import run_bass_kernel_spmd

S = 65536
H = 1024
N_CORES = 8
SC = S // N_CORES          # 8192 rows per core
P = 128                    # partitions
NT = SC // P               # 64 blocks of 128 rows per core
GMAX = 4                   # max blocks per DMA chunk (2 MB; 8-block
                           # chunks hit a DVE efficiency cliff)

# chunk sizes in blocks; tapered at the end (shorter post-DMA tail)
CHUNKS = [4] * 15 + [2, 1, 1]
assert sum(CHUNKS) == NT

INP_BUFS = 6

_DT = mybir.dt.float32


def _vector_blocks(g, nb):
    """Block offsets (within chunk g) whose row-sum runs on the DVE;
    the rest run on the Scalar engine.  The DVE carries all the
    multiplies, so mid-stream it only takes a few; at the taper it
    takes the blocks interleaved with its final muls while the Scalar
    engine drains chunks 14/15 in parallel."""
    if nb == 4:
        return [3] if g in (1, 3, 5, 7, 9) else []
    if nb == 2:
        return [1]            # chunk 15: [s, v]
    return [0]                # chunks 16, 17: DVE


def _build_nc():
    nc = bacc.Bacc("TRN2", target_bir_lowering=False, debug=False,
                   enable_asserts=False, num_devices=N_CORES)
    enc = nc.dram_tensor("enc", [SC, H], _DT, kind="ExternalInput")
    hid = nc.dram_tensor("hid", [P, H], _DT, kind="ExternalInput")
    n_v = sum(len(_vector_blocks(g, nb)) for g, nb in enumerate(CHUNKS))
    out_s = nc.dram_tensor("out_s", [P, NT], _DT, kind="ExternalOutput")
    out_v = nc.dram_tensor("out_v", [P, n_v], _DT, kind="ExternalOutput")

    with tile.TileContext(nc) as tc:
        with (
            tc.tile_pool(name="inp", bufs=INP_BUFS) as inp_pool,
            tc.tile_pool(name="small", bufs=1) as small,
        ):
            hidrep = small.tile([P, H], _DT)

            # separate tiles per engine so the Tile tracker never makes
            # one engine's energy writes wait on the other's
            energies_s = small.tile([P, NT], _DT)
            energies_v = small.tile([P, max(n_v, 1)], _DT)
            # the DVE-reduced columns of energies_s are never written on
            # device (host overwrites them from out_v); zero once so the
            # final out_s DMA reads initialized memory
            nc.gpsimd.memset(energies_s[:], 0.0)

            vcol = 0
            blk = 0
            for g, nb in enumerate(CHUNKS):
                r0 = blk * P
                t_in = inp_pool.tile([P, GMAX * H], _DT, tag="t_in")
                # partition p <- rows [r0 + p*nb, r0 + (p+1)*nb):
                # 4*nb KB contiguous per partition.  Alternate between
                # the two HWDGE rings.
                dma_eng = nc.sync if g % 2 == 0 else nc.scalar
                dma_eng.dma_start(
                    t_in[:, :nb * H].rearrange("p (b h) -> p b h", h=H),
                    enc.ap()[r0:r0 + nb * P, :].rearrange(
                        "(p b) h -> p b h", p=P),
                )
                if g == 0:
                    # issued after chunk 0 so the big stream's first
                    # byte isn't delayed behind the hid replica
                    nc.scalar.dma_start(hidrep[:], hid.ap())
                hid_bc = hidrep[:].rearrange(
                    "p (o h) -> p o h", o=1).broadcast_to((P, nb, H))
                nc.vector.tensor_mul(
                    t_in[:, :nb * H].rearrange("p (b h) -> p b h", h=H),
                    t_in[:, :nb * H].rearrange("p (b h) -> p b h", h=H),
                    hid_bc,
                )
                vblocks = _vector_blocks(g, nb)
                for j in range(nb):
                    seg = t_in[:, j * H:(j + 1) * H]
                    if j in vblocks:
                        nc.vector.reduce_sum(
                            energies_v[:, vcol:vcol + 1], seg,
                            axis=mybir.AxisListType.X,
                        )
                        vcol += 1
                    else:
                        nc.scalar.activation(
                            seg, seg,
                            mybir.ActivationFunctionType.Identity,
                            accum_out=energies_s[:, blk + j:blk + j + 1],
                        )
                blk += nb

            nc.sync.dma_start(out_s.ap(), energies_s[:])
            nc.sync.dma_start(out_v.ap(), energies_v[:, :n_v])
    nc.compile()
    return nc


_NC_CACHE = None


def _get_nc():
    global _NC_CACHE
    if _NC_CACHE is None:
        _NC_CACHE = _build_nc()
    return _NC_CACHE


def run_device(hidden, encoder_outputs, **spmd_kwargs):
    """Run the per-core kernels; returns (list of per-core result dicts,
    BassKernelResults)."""
    hidden = np.asarray(hidden, dtype=np.float32)
    encoder_outputs = np.asarray(encoder_outputs, dtype=np.float32)
    hidrep = np.ascontiguousarray(np.broadcast_to(hidden, (P, H)))
    in_maps = [
        {
            "enc": np.ascontiguousarray(encoder_outputs[c * SC:(c + 1) * SC]),
            "hid": hidrep,
        }
        for c in range(N_CORES)
    ]
    # The axon-proxied runtime occasionally reports the accelerator as
    # unrecoverable and then recovers on the next attempt; retry.
    last_err = None
    for attempt in range(3):
        try:
            res = run_bass_kernel_spmd(
                _get_nc(), in_maps, list(range(N_CORES)), **spmd_kwargs
            )
            return res.results, res
        except Exception as e:  # noqa: BLE001
            last_err = e
            time.sleep(2.0)
    raise last_err


def _maps():
    """(vcols, perm): vcols[i] = global block column of the i-th DVE
    reduce; perm[s_local] = flat index into the merged [P, NT] energies."""
    vcols = []
    blk = 0
    for g, nb in enumerate(CHUNKS):
        for j in _vector_blocks(g, nb):
            vcols.append(blk + j)
        blk += nb
    perm = np.empty(SC, dtype=np.int64)
    blk = 0
    for nb in CHUNKS:
        r0 = blk * P
        for p in range(P):
            base = r0 + p * nb
            for j in range(nb):
                perm[base + j] = p * NT + (blk + j)
        blk += nb
    return np.array(vcols), perm


_VCOLS, _PERM = _maps()


def combine(results):
    """Host-side softmax over the gathered energies -> [1, 1, S] f32."""
    es = []
    for r in results:
        E = r["out_s"].copy()
        E[:, _VCOLS] = r["out_v"]
        es.append(E.reshape(P * NT)[_PERM])
    e = np.concatenate(es).astype(np.float64)
    e -= e.max()
    x = np.exp(e)
    attn = x / x.sum()
    return attn.astype(np.float32)[None, None, :]


def kernel(hidden, encoder_outputs):
    results, _ = run_device(hidden, encoder_outputs)
    return combine(results)


# revision 17
# speedup vs baseline: 1.4955x; 1.3351x over previous
"""Trainium2 Bass kernel for nn_Attn_14078902796904.

Computes attn = softmax(encoder_outputs @ hidden) for
encoder_outputs [65536, 1024] f32, hidden [1024] f32 -> [1, 1, 65536] f32.

Strategy (sequence-parallel across 8 NeuronCores):
  - Core c gets rows [c*8192, (c+1)*8192) of encoder_outputs; hidden is
    replicated (host pre-broadcasts it to [128, 1024]).
  - On-core: stream the 32 MB shard through SBUF in [128, nb*1024]
    chunks with a CONTIGUOUS per-partition layout (partition p of a
    chunk holds nb consecutive rows -> 16 KB contiguous HBM reads per
    partition, 16 KB DMA descriptors, ~405 GB/s observed).
    All chunks keep the full 128-partition shape: partition-sliced DMAs
    collapse onto 4 of the 16 SDMA rings (~1/4 bandwidth -- measured).
  - Compute: the DVE multiplies each chunk by hid IN PLACE (f32
    tensor_tensor is 1 elem/lane/cycle -> 4.42 us per 4-block chunk vs
    the 5.2-5.9 us DMA pace).  Per-block row-sums are split between the
    Scalar engine (activation Identity + accum_out, 1.22 us/block
    sustained) and the DVE (reduce_sum, 1.22 us/block): scalar takes
    most blocks, the DVE takes a few mid-stream ones plus the tapered
    tail blocks interleaved with its final muls, so neither engine
    holds a backlog when the stream ends.
  - Device returns raw energies; softmax runs on the host in float64
    over all 65536 gathered energies.  No on-device softmax -> no
    ACT_TABLE_LOAD, minimal post-stream tail.
  (tensor_tensor_reduce would fuse mul+reduce in one DVE op but crashes
  the execution unit on this runtime path -- probed 2026-08-08.)
"""

import os
import sys
import time

for _p in ("/opt/trn_rl_repo", "/root/.axon_site/_ro/trn_rl_repo"):
    if os.path.isdir(_p) and _p not in sys.path:
        sys.path.append(_p)

import numpy as np

import concourse.tile as tile
from concourse import bacc, mybir
from concourse.bass_utils import run_bass_kernel_spmd

S = 65536
H = 1024
N_CORES = 8
SC = S // N_CORES          # 8192 rows per core
P = 128                    # partitions
NT = SC // P               # 64 blocks of 128 rows per core
GMAX = 4                   # max blocks per DMA chunk (2 MB; 8-block
                           # chunks hit a DVE efficiency cliff)

# chunk sizes in blocks; tapered at the end (shorter post-DMA tail)
CHUNKS = [4] * 15 + [2, 1, 1]
assert sum(CHUNKS) == NT

INP_BUFS = 6

_DT = mybir.dt.float32


def _vector_blocks(g, nb):
    """Block offsets (within chunk g) whose row-sum runs on the DVE;
    the rest run on the Scalar engine.  The DVE carries all the
    multiplies, so mid-stream it only takes a few; at the taper it
    takes the blocks interleaved with its final muls while the Scalar
    engine drains chunks 14/15 in parallel."""
    if nb == 4:
        return [3] if g in (1, 3, 5, 7, 9) else []
    if nb == 2:
        return [1]            # chunk 15: [s, v]
    return [0]                # chunks 16, 17: DVE


def _build_nc():
    nc = bacc.Bacc("TRN2", target_bir_lowering=False, debug=False,
                   enable_asserts=False, num_devices=N_CORES)
    enc = nc.dram_tensor("enc", [SC, H], _DT, kind="ExternalInput")
    hid = nc.dram_tensor("hid", [P, H], _DT, kind="ExternalInput")
    n_v = sum(len(_vector_blocks(g, nb)) for g, nb in enumerate(CHUNKS))
    out_s = nc.dram_tensor("out_s", [P, NT], _DT, kind="ExternalOutput")
    out_v = nc.dram_tensor("out_v", [P, n_v], _DT, kind="ExternalOutput")

    with tile.TileContext(nc) as tc:
        with (
            tc.tile_pool(name="inp", bufs=INP_BUFS) as inp_pool,
            tc.tile_pool(name="small", bufs=1) as small,
        ):
            hidrep = small.tile([P, H], _DT)

            # separate tiles per engine so the Tile tracker never makes
            # one engine's energy writes wait on the other's
            energies_s = small.tile([P, NT], _DT)
            energies_v = small.tile([P, max(n_v, 1)], _DT)
            # the DVE-reduced columns of energies_s are never written on
            # device (host overwrites them from out_v); zero once so the
            # final out_s DMA reads initialized memory
            nc.gpsimd.memset(energies_s[:], 0.0)

            vcol = 0
            blk = 0
            for g, nb in enumerate(CHUNKS):
                r0 = blk * P
                t_in = inp_pool.tile([P, GMAX * H], _DT, tag="t_in")
                # partition p <- rows [r0 + p*nb, r0 + (p+1)*nb):
                # 4*nb KB contiguous per partition.  All chunk DMAs
                # issue from the Sync sequencer: scalar-issued DMAs
                # serialize behind the Scalar engine's ACTIVATE queue
                # and starve the stream (measured +15 us).
                nc.sync.dma_start(
                    t_in[:, :nb * H].rearrange("p (b h) -> p b h", h=H),
                    enc.ap()[r0:r0 + nb * P, :].rearrange(
                        "(p b) h -> p b h", p=P),
                )
                if g == 0:
                    # issued after chunk 0 so the big stream's first
                    # byte isn't delayed behind the hid replica
                    nc.scalar.dma_start(hidrep[:], hid.ap())  # scalar is idle during ramp
                hid_bc = hidrep[:].rearrange(
                    "p (o h) -> p o h", o=1).broadcast_to((P, nb, H))
                nc.vector.tensor_mul(
                    t_in[:, :nb * H].rearrange("p (b h) -> p b h", h=H),
                    t_in[:, :nb * H].rearrange("p (b h) -> p b h", h=H),
                    hid_bc,
                )
                vblocks = _vector_blocks(g, nb)
                for j in range(nb):
                    seg = t_in[:, j * H:(j + 1) * H]
                    if j in vblocks:
                        nc.vector.reduce_sum(
                            energies_v[:, vcol:vcol + 1], seg,
                            axis=mybir.AxisListType.X,
                        )
                        vcol += 1
                    else:
                        nc.scalar.activation(
                            seg, seg,
                            mybir.ActivationFunctionType.Identity,
                            accum_out=energies_s[:, blk + j:blk + j + 1],
                        )
                blk += nb

            nc.sync.dma_start(out_s.ap(), energies_s[:])
            nc.sync.dma_start(out_v.ap(), energies_v[:, :n_v])
    nc.compile()
    return nc


_NC_CACHE = None


def _get_nc():
    global _NC_CACHE
    if _NC_CACHE is None:
        _NC_CACHE = _build_nc()
    return _NC_CACHE


def run_device(hidden, encoder_outputs, **spmd_kwargs):
    """Run the per-core kernels; returns (list of per-core result dicts,
    BassKernelResults)."""
    hidden = np.asarray(hidden, dtype=np.float32)
    encoder_outputs = np.asarray(encoder_outputs, dtype=np.float32)
    hidrep = np.ascontiguousarray(np.broadcast_to(hidden, (P, H)))
    in_maps = [
        {
            "enc": np.ascontiguousarray(encoder_outputs[c * SC:(c + 1) * SC]),
            "hid": hidrep,
        }
        for c in range(N_CORES)
    ]
    # The axon-proxied runtime occasionally reports the accelerator as
    # unrecoverable and then recovers on the next attempt; retry.
    last_err = None
    for attempt in range(3):
        try:
            res = run_bass_kernel_spmd(
                _get_nc(), in_maps, list(range(N_CORES)), **spmd_kwargs
            )
            return res.results, res
        except Exception as e:  # noqa: BLE001
            last_err = e
            time.sleep(2.0)
    raise last_err


def _maps():
    """(vcols, perm): vcols[i] = global block column of the i-th DVE
    reduce; perm[s_local] = flat index into the merged [P, NT] energies."""
    vcols = []
    blk = 0
    for g, nb in enumerate(CHUNKS):
        for j in _vector_blocks(g, nb):
            vcols.append(blk + j)
        blk += nb
    perm = np.empty(SC, dtype=np.int64)
    blk = 0
    for nb in CHUNKS:
        r0 = blk * P
        for p in range(P):
            base = r0 + p * nb
            for j in range(nb):
                perm[base + j] = p * NT + (blk + j)
        blk += nb
    return np.array(vcols), perm


_VCOLS, _PERM = _maps()


def combine(results):
    """Host-side softmax over the gathered energies -> [1, 1, S] f32."""
    es = []
    for r in results:
        E = r["out_s"].copy()
        E[:, _VCOLS] = r["out_v"]
        es.append(E.reshape(P * NT)[_PERM])
    e = np.concatenate(es).astype(np.float64)
    e -= e.max()
    x = np.exp(e)
    attn = x / x.sum()
    return attn.astype(np.float32)[None, None, :]


def kernel(hidden, encoder_outputs):
    results, _ = run_device(hidden, encoder_outputs)
    return combine(results)
